# revision 19
# baseline (speedup 1.0000x reference)
"""Trainium2 Bass kernel for nn_CombineUV (shortlist-scored retrieval).

Math: out[b,s] = dot(input[b], sig(alpha)*weight[i] + sig(beta)*labels[i]) + bias[i]
with i = shortlist[b,s].  The gate is a host-side scalar row-scale, so the
combined table TC = sig(alpha)*weight + sig(beta)*labels is materialized ONCE
on the host as [L, 512] bf16 — halving both HBM traffic and PE work vs
streaming weight and labels separately.

Device strategy (8 cores, L-sharded, segment-packed streams):
 - Core c owns table rows [c*16384, (c+1)*16384). Each (b,s) pair routes to
   the core owning its row.
 - Per core, pairs are grouped into SEGMENTS: up to 2 hits of one row whose
   batches fit a 112-wide window anchored at the first hit. Each segment is
   one column of a [128, 4*512] bf16 PE-ready tile, host-pregathered and
   loaded with a plain full-rate dma_start. Duplicate hits therefore SHARE
   one streamed copy of their row whenever their batches are close.
 - Tiles are packed JOINTLY across cores (the per-tile batch-window base is
   compiled into the shared SPMD program): each round the window is set by
   the slowest core's next segment and every core fills the columns that fit.
 - Per tile: 4 accumulating matmuls with lhsT = XC[:, c, blo:+128] give
   PSUM[m, j] = x[blo+m] . TC[row_j]; DVE pass 1 multiplies a host-built
   one-hot mask (selects m1_j per column), PE reduces partitions with a
   ones-vector matmul. Columns with a second hit sit first in the tile, and
   a prefix-width pass 2 (mask2/reduce) serves them — no extra DMA.
 - Reduce outputs land on rows {0,32,64} of a shared PSUM bank (matmul base
   partitions must be 0/32/64); one scalar-engine copy + one strided DMA
   ships 3 results at once.
 - Host adds bias[shortlist] (O(B*S) elementwise) and inverse-permutes.
"""

import sys

sys.path.insert(0, "/opt/trn_rl_repo")

import numpy as np
import ml_dtypes

BF16 = ml_dtypes.bfloat16

L, D, B, S = 131072, 512, 512, 512
NCORES = 8
LSH = L // NCORES          # table rows per core
TILE = 512                 # columns (segments) per tile
MWIN = 128                 # batch-window width for the lhsT slice
MW_SEG = 112               # per-segment batch span (128-MW_SEG anchor drift)
CAP = 2                    # max hits per column -> max 2 select passes
NCHUNK = D // 128          # 4 chunks of 128 along the combined-row axis

_PROG_CACHE = {}


def _segment_core(li, bv, pos, cap):
    """Greedy segmentation of one core's pairs; anchor-sorted output."""
    o = np.lexsort((bv, li))
    li, bv, pos = li[o], bv[o], pos[o]
    n = len(li)
    seg_row, seg_b, seg_pos = [], [], []
    i = 0
    while i < n:
        r = li[i]
        j = i
        bs, ps = [], []
        while j < n and li[j] == r and len(bs) < cap and (
            not bs or bv[j] - bs[0] < MW_SEG
        ):
            bs.append(int(bv[j]))
            ps.append(int(pos[j]))
            j += 1
        seg_row.append(r)
        seg_b.append(bs)
        seg_pos.append(ps)
        i = j
    anchor = np.array([b[0] for b in seg_b], np.int64)
    order = np.argsort(anchor, kind="stable")
    return (
        np.array(seg_row, np.int64)[order],
        [seg_b[s] for s in order],
        [seg_pos[s] for s in order],
    )


def _joint_pack(cores_segs):
    """Shared per-tile window base across all cores; greedy fill."""
    NC = len(cores_segs)
    ptr = [0] * NC
    nseg = [len(cs[0]) for cs in cores_segs]
    blo = []
    tiles = [[] for _ in range(NC)]
    while any(ptr[c] < nseg[c] for c in range(NC)):
        w = min(
            cores_segs[c][1][ptr[c]][0] for c in range(NC) if ptr[c] < nseg[c]
        )
        w = min(w, B - MWIN)
        blo.append(w)
        for c in range(NC):
            _, seg_b, _ = cores_segs[c]
            s0 = ptr[c]
            s = s0
            while (
                s < nseg[c]
                and s - s0 < TILE
                and seg_b[s][0] >= w
                and seg_b[s][-1] - w < MWIN
            ):
                s += 1
            tiles[c].append((s0, s))
            ptr[c] = s
    return np.array(blo, np.int64), tiles


def _build_plans(sl, cap):
    core = sl // LSH
    lidx = sl % LSH
    bvec = np.repeat(np.arange(B, dtype=np.int64), S)
    cores_segs = []
    for c in range(NCORES):
        posc = np.nonzero(core == c)[0]
        cores_segs.append(_segment_core(lidx[posc], bvec[posc], posc, cap))
    blo, tiles = _joint_pack(cores_segs)
    ntiles = len(blo)
    plans = []
    w2 = np.zeros(ntiles, np.int64)
    for c in range(NCORES):
        rows_s, seg_b, seg_pos = cores_segs[c]
        rows = np.zeros((ntiles, TILE), np.int64)
        m1 = np.full((ntiles, TILE), -1, np.int64)
        m2 = np.full((ntiles, TILE), -1, np.int64)
        p1 = np.full((ntiles, TILE), -1, np.int64)
        p2 = np.full((ntiles, TILE), -1, np.int64)
        n2 = np.zeros(ntiles, np.int64)
        for t, (s0, s1) in enumerate(tiles[c]):
            segs = sorted(range(s0, s1), key=lambda s: -len(seg_b[s]))
            bl = blo[t]
            for j, s in enumerate(segs):
                rows[t, j] = rows_s[s]
                m1[t, j] = seg_b[s][0] - bl
                p1[t, j] = seg_pos[s][0]
                if len(seg_b[s]) > 1:
                    m2[t, j] = seg_b[s][1] - bl
                    p2[t, j] = seg_pos[s][1]
            n2[t] = sum(1 for s in segs if len(seg_b[s]) > 1)
            ncols = s1 - s0
            if ncols and not (
                (m1[t, :ncols] >= 0).all() and (m1[t, :ncols] < MWIN).all()
            ):
                return None
            if n2[t] and not (
                (m2[t, : n2[t]] >= 0).all() and (m2[t, : n2[t]] < MWIN).all()
            ):
                return None
        w2 = np.maximum(w2, n2)
        plans.append({"rows": rows, "m1": m1, "m2": m2, "p1": p1, "p2": p2})
    # Heaviest pass-2 tiles first: the end of the schedule is compute-paced
    # (DMA finishes early), so make the tail tiles cheap.
    perm = np.argsort(-w2, kind="stable")
    blo = blo[perm]
    w2 = w2[perm]
    for pl in plans:
        for k in pl:
            pl[k] = pl[k][perm]
    return blo, ntiles, w2, plans


def _build_program(ntiles, blo, w2):
    import concourse.bacc as bacc
    import concourse.mybir as mybir
    from concourse.tile import TileContext

    f32, bf = mybir.dt.float32, mybir.dt.bfloat16
    u8 = mybir.dt.uint8
    off2 = np.concatenate([[0], np.cumsum(w2)])
    w2tot = int(off2[-1])
    nred = ntiles + int((w2 > 0).sum())
    ngrp = -(-nred // 3)

    nc = bacc.Bacc(None, target_bir_lowering=False)
    st_d = nc.dram_tensor(
        "stream", [ntiles, 128, NCHUNK * TILE], bf, kind="ExternalInput"
    )
    xc_d = nc.dram_tensor("xc", [128, NCHUNK * B], bf, kind="ExternalInput")
    mask_d = nc.dram_tensor("mask", [MWIN, ntiles * TILE], u8, kind="ExternalInput")
    mask2_d = nc.dram_tensor("mask2", [MWIN, max(w2tot, 1)], u8, kind="ExternalInput")
    ones_d = nc.dram_tensor("ones", [MWIN, 1], bf, kind="ExternalInput")
    out_d = nc.dram_tensor("out", [3, ngrp * TILE], f32, kind="ExternalOutput")

    MCH = 8  # mask chunk granularity (tiles) for just-in-time mask loads

    with TileContext(nc) as tc:
        with (
            tc.tile_pool(name="res", bufs=1) as res_pool,
            tc.tile_pool(name="g", bufs=14) as gpool,
            tc.tile_pool(name="m", bufs=12) as mpool,
            tc.tile_pool(name="o", bufs=4) as opool,
            tc.tile_pool(name="ps", bufs=5, space="PSUM") as pspool,
            tc.tile_pool(name="pso", bufs=2, space="PSUM") as psopool,
        ):
            xc_sb = res_pool.tile([128, NCHUNK * B], bf, tag="xc")
            nc.sync.dma_start(out=xc_sb[:], in_=xc_d[:])
            ones_sb = res_pool.tile([MWIN, 1], bf, tag="ones")
            nc.sync.dma_start(out=ones_sb[:], in_=ones_d[:])
            # mask/mask2 SBUF tiles are resident but loaded just-in-time in
            # MCH-tile chunks so the stream loads aren't stuck behind them.
            mask_sb = res_pool.tile([MWIN, ntiles * TILE], u8, tag="mask")
            mask2_sb = res_pool.tile([MWIN, max(w2tot, 1)], u8, tag="mask2")

            DELAY = 7  # pending masked tiles between DVE mask-mult and PE reduce
            pending = []
            state = {"r": 0, "bank": None}

            def emit_reduce(msk_u, width):
                r = state["r"]
                q, row = divmod(r, 3)
                if row == 0:
                    state["bank"] = psopool.tile(
                        [128, TILE], f32, tag="pso", name="pso_bank"
                    )
                bank = state["bank"]
                nc.tensor.matmul(
                    out=bank[32 * row : 32 * row + 1, :width],
                    lhsT=ones_sb[:],
                    rhs=msk_u[:, :width],
                    start=True,
                    stop=True,
                )
                if row == 2 or r == nred - 1:
                    # Ship this bank with one ACT copy of partitions 0..64
                    # (only rows 0/32/64 carry results; compute engines
                    # cannot stride partitions, DMA can).
                    nrow = row + 1
                    span = 32 * (nrow - 1) + 1
                    ot = opool.tile([65, TILE], f32, tag="ot", name="ot_buf")
                    nc.scalar.copy(ot[:span, :], bank[:span, :])
                    nc.scalar.dma_start(
                        out=out_d[:nrow, q * TILE : (q + 1) * TILE],
                        in_=ot[:span:32, :],
                    )
                state["r"] = r + 1

            for t in range(ntiles):
                bl = int(blo[t])
                if t % MCH == 0:
                    # JIT mask chunk for tiles [t, t+MCH)
                    lo, hi = t * TILE, min(ntiles, t + MCH) * TILE
                    nc.sync.dma_start(
                        out=mask_sb[:, lo:hi], in_=mask_d[:, lo:hi]
                    )
                    l2, h2 = int(off2[t]), int(off2[min(ntiles, t + MCH)])
                    if h2 > l2:
                        nc.scalar.dma_start(
                            out=mask2_sb[:, l2:h2], in_=mask2_d[:, l2:h2]
                        )
                g = gpool.tile([128, NCHUNK * TILE], bf, tag="g")
                eng = nc.sync if t % 2 == 0 else nc.scalar
                eng.dma_start(out=g[:], in_=st_d[t])
                ps = pspool.tile([MWIN, TILE], f32, tag="ps")
                for c in range(NCHUNK):
                    nc.tensor.matmul(
                        out=ps[:],
                        lhsT=xc_sb[:, c * B + bl : c * B + bl + MWIN],
                        rhs=g[:, c * TILE : (c + 1) * TILE],
                        start=(c == 0),
                        stop=(c == NCHUNK - 1),
                    )
                while len(pending) > DELAY:
                    emit_reduce(*pending.pop(0))
                msk = mpool.tile([MWIN, TILE], bf, tag="msk")
                nc.vector.tensor_tensor(
                    out=msk[:],
                    in0=ps[:],
                    in1=mask_sb[:, t * TILE : (t + 1) * TILE],
                    op=mybir.AluOpType.mult,
                )
                pending.append((msk, TILE))
                wt = int(w2[t])
                if wt > 0:
                    # Pass 2: serves each column's second hit — the 2-hit
                    # columns sit first, so only a prefix is touched.
                    msk2 = mpool.tile([MWIN, TILE], bf, tag="msk2")
                    nc.vector.tensor_tensor(
                        out=msk2[:, :wt],
                        in0=ps[:, :wt],
                        in1=mask2_sb[:, int(off2[t]) : int(off2[t]) + wt],
                        op=mybir.AluOpType.mult,
                    )
                    pending.append((msk2, wt))
            for args in pending:
                emit_reduce(*args)

    nc.compile()
    return nc


def _prep_inputs(input, labels, weight, alpha, beta, shortlist, cap=CAP):
    """Host-side staging: gate fold into a single [L,512] bf16 table, segment
    packing, per-tile pre-transpose, mask build."""
    input = np.asarray(input, dtype=np.float32)
    alpha = np.asarray(alpha, dtype=np.float32).reshape(1, D)
    beta = np.asarray(beta, dtype=np.float32).reshape(1, D)
    sa = 1.0 / (1.0 + np.exp(-alpha))
    sb = 1.0 / (1.0 + np.exp(-beta))

    # XC[p, c, b]: chunk c of input for batch b.
    XC = np.ascontiguousarray(
        input.T.reshape(NCHUNK, 128, B).transpose(1, 0, 2)
    ).astype(BF16)

    TC = (
        np.asarray(weight, np.float32) * sa + np.asarray(labels, np.float32) * sb
    ).astype(BF16)  # [L, 512]

    sl = np.asarray(shortlist).reshape(-1).astype(np.int64)
    built = _build_plans(sl, cap)
    if built is None:
        return None
    blo, ntiles, w2, plans = built
    off2 = np.concatenate([[0], np.cumsum(w2)])
    w2tot = int(off2[-1])

    in_maps = []
    ones = np.ones((MWIN, 1), dtype=BF16)
    xc_flat = np.ascontiguousarray(XC.reshape(128, NCHUNK * B))
    for c in range(NCORES):
        pl = plans[c]
        arr = TC[c * LSH : (c + 1) * LSH][pl["rows"].reshape(-1)]
        arr = arr.reshape(ntiles, TILE, NCHUNK, 128)      # [t, j, ch, p]
        stream = np.ascontiguousarray(arr.transpose(0, 3, 2, 1)).reshape(
            ntiles, 128, NCHUNK * TILE
        )
        maskh = np.zeros((MWIN, ntiles * TILE), np.uint8)
        mask2h = np.zeros((MWIN, max(w2tot, 1)), np.uint8)
        tt, jj = np.nonzero(pl["m1"] >= 0)
        maskh[pl["m1"][tt, jj], tt * TILE + jj] = 1
        tt, jj = np.nonzero(pl["m2"] >= 0)
        mask2h[pl["m2"][tt, jj], off2[tt] + jj] = 1
        in_maps.append(
            {
                "stream": stream,
                "xc": xc_flat,
                "mask": maskh,
                "mask2": mask2h,
                "ones": ones,
            }
        )
    meta = {"blo": blo, "ntiles": ntiles, "w2": w2, "off2": off2, "plans": plans}
    return in_maps, meta


def kernel(input, labels, weight, alpha, beta, bias, shortlist, _trace=False):
    from concourse.bass_utils import run_bass_kernel_spmd

    prep = _prep_inputs(input, labels, weight, alpha, beta, shortlist)
    if prep is None:
        # Window invariant violated (cannot happen by construction, but keep
        # a safe fallback): one hit per column, pure batch-sorted packing.
        prep = _prep_inputs(input, labels, weight, alpha, beta, shortlist, cap=1)
    assert prep is not None, "segment packing failed"
    in_maps, meta = prep
    ntiles, w2, blo = meta["ntiles"], meta["w2"], meta["blo"]

    key = (ntiles, tuple(int(x) for x in w2), tuple(int(x) for x in blo))
    if key not in _PROG_CACHE:
        _PROG_CACHE[key] = _build_program(ntiles, blo, w2)
    nc = _PROG_CACHE[key]

    res = run_bass_kernel_spmd(nc, in_maps, list(range(NCORES)), trace=_trace)

    # Reduce r (emission order: per tile, pass1 then pass2-if-any) lives at
    # out[r%3, (r//3)*TILE : +TILE].
    r1 = np.empty(ntiles, np.int64)
    r2 = np.full(ntiles, -1, np.int64)
    r = 0
    for t in range(ntiles):
        r1[t] = r
        r += 1
        if w2[t] > 0:
            r2[t] = r
            r += 1

    out_flat = np.zeros(B * S, dtype=np.float32)
    for c in range(NCORES):
        o = res.results[c]["out"]  # [3, ngrp*TILE]
        pl = meta["plans"][c]
        for t in range(ntiles):
            q, row = divmod(int(r1[t]), 3)
            vals = o[row, q * TILE : (q + 1) * TILE]
            sel = pl["p1"][t] >= 0
            out_flat[pl["p1"][t][sel]] = vals[sel]
            if r2[t] >= 0:
                q, row = divmod(int(r2[t]), 3)
                vals = o[row, q * TILE : (q + 1) * TILE]
                sel = pl["p2"][t] >= 0
                out_flat[pl["p2"][t][sel]] = vals[sel]

    bias = np.asarray(bias, dtype=np.float32)
    sl = np.asarray(shortlist).reshape(-1).astype(np.int64)
    out_flat += bias[sl]
    out = out_flat.reshape(B, S)

    if _trace:
        return out, res
    return out


# revision 22
# speedup vs baseline: 1.0204x; 1.0204x over previous
"""Trainium2 Bass kernel for nn_CombineUV (shortlist-scored retrieval).

Math: out[b,s] = dot(input[b], sig(alpha)*weight[i] + sig(beta)*labels[i]) + bias[i]
with i = shortlist[b,s].  The gate is a host-side scalar row-scale, so the
combined table TC = sig(alpha)*weight + sig(beta)*labels is materialized ONCE
on the host as [L, 512] bf16 — halving both HBM traffic and PE work vs
streaming weight and labels separately.

Device strategy (8 cores, L-sharded, segment-packed streams):
 - Core c owns table rows [c*16384, (c+1)*16384). Each (b,s) pair routes to
   the core owning its row.
 - Per core, pairs are grouped into SEGMENTS: up to 2 hits of one row whose
   batches fit a 112-wide window anchored at the first hit. Each segment is
   one column of a [128, 4*512] bf16 PE-ready tile, host-pregathered and
   loaded with a plain full-rate dma_start. Duplicate hits therefore SHARE
   one streamed copy of their row whenever their batches are close.
 - Tiles are packed JOINTLY across cores (the per-tile batch-window base is
   compiled into the shared SPMD program): each round the window is set by
   the slowest core's next segment and every core fills the columns that fit.
 - Per tile: 4 accumulating matmuls with lhsT = XC[:, c, blo:+128] give
   PSUM[m, j] = x[blo+m] . TC[row_j]; DVE pass 1 multiplies a host-built
   one-hot mask (selects m1_j per column), PE reduces partitions with a
   ones-vector matmul. Columns with a second hit sit first in the tile, and
   a prefix-width pass 2 (mask2/reduce) serves them — no extra DMA.
 - Reduce outputs land on rows {0,32,64} of a shared PSUM bank (matmul base
   partitions must be 0/32/64); one scalar-engine copy + one strided DMA
   ships 3 results at once.
 - Host adds bias[shortlist] (O(B*S) elementwise) and inverse-permutes.
"""

import sys

sys.path.insert(0, "/opt/trn_rl_repo")

import numpy as np
import ml_dtypes

BF16 = ml_dtypes.bfloat16

L, D, B, S = 131072, 512, 512, 512
NCORES = 8
LSH = L // NCORES          # table rows per core
TILE = 512                 # columns (segments) per tile
MWIN = 128                 # batch-window width for the lhsT slice
MW_SEG = 112               # per-segment batch span (128-MW_SEG anchor drift)
CAP = 2                    # max hits per column -> max 2 select passes
NCHUNK = D // 128          # 4 chunks of 128 along the combined-row axis

_PROG_CACHE = {}


def _segment_core(li, bv, pos, cap):
    """Greedy segmentation of one core's pairs; anchor-sorted output."""
    o = np.lexsort((bv, li))
    li, bv, pos = li[o], bv[o], pos[o]
    n = len(li)
    seg_row, seg_b, seg_pos = [], [], []
    i = 0
    while i < n:
        r = li[i]
        j = i
        bs, ps = [], []
        while j < n and li[j] == r and len(bs) < cap and (
            not bs or bv[j] - bs[0] < MW_SEG
        ):
            bs.append(int(bv[j]))
            ps.append(int(pos[j]))
            j += 1
        seg_row.append(r)
        seg_b.append(bs)
        seg_pos.append(ps)
        i = j
    anchor = np.array([b[0] for b in seg_b], np.int64)
    order = np.argsort(anchor, kind="stable")
    return (
        np.array(seg_row, np.int64)[order],
        [seg_b[s] for s in order],
        [seg_pos[s] for s in order],
    )


def _joint_pack(cores_segs):
    """Shared per-tile window base across all cores; greedy fill."""
    NC = len(cores_segs)
    ptr = [0] * NC
    nseg = [len(cs[0]) for cs in cores_segs]
    blo = []
    tiles = [[] for _ in range(NC)]
    while any(ptr[c] < nseg[c] for c in range(NC)):
        w = min(
            cores_segs[c][1][ptr[c]][0] for c in range(NC) if ptr[c] < nseg[c]
        )
        w = min(w, B - MWIN)
        blo.append(w)
        for c in range(NC):
            _, seg_b, _ = cores_segs[c]
            s0 = ptr[c]
            s = s0
            while (
                s < nseg[c]
                and s - s0 < TILE
                and seg_b[s][0] >= w
                and seg_b[s][-1] - w < MWIN
            ):
                s += 1
            tiles[c].append((s0, s))
            ptr[c] = s
    return np.array(blo, np.int64), tiles


def _build_plans(sl, cap):
    core = sl // LSH
    lidx = sl % LSH
    bvec = np.repeat(np.arange(B, dtype=np.int64), S)
    cores_segs = []
    for c in range(NCORES):
        posc = np.nonzero(core == c)[0]
        cores_segs.append(_segment_core(lidx[posc], bvec[posc], posc, cap))
    blo, tiles = _joint_pack(cores_segs)
    ntiles = len(blo)
    plans = []
    w2 = np.zeros(ntiles, np.int64)
    for c in range(NCORES):
        rows_s, seg_b, seg_pos = cores_segs[c]
        rows = np.zeros((ntiles, TILE), np.int64)
        m1 = np.full((ntiles, TILE), -1, np.int64)
        m2 = np.full((ntiles, TILE), -1, np.int64)
        p1 = np.full((ntiles, TILE), -1, np.int64)
        p2 = np.full((ntiles, TILE), -1, np.int64)
        n2 = np.zeros(ntiles, np.int64)
        for t, (s0, s1) in enumerate(tiles[c]):
            segs = sorted(range(s0, s1), key=lambda s: -len(seg_b[s]))
            bl = blo[t]
            for j, s in enumerate(segs):
                rows[t, j] = rows_s[s]
                m1[t, j] = seg_b[s][0] - bl
                p1[t, j] = seg_pos[s][0]
                if len(seg_b[s]) > 1:
                    m2[t, j] = seg_b[s][1] - bl
                    p2[t, j] = seg_pos[s][1]
            n2[t] = sum(1 for s in segs if len(seg_b[s]) > 1)
            ncols = s1 - s0
            if ncols and not (
                (m1[t, :ncols] >= 0).all() and (m1[t, :ncols] < MWIN).all()
            ):
                return None
            if n2[t] and not (
                (m2[t, : n2[t]] >= 0).all() and (m2[t, : n2[t]] < MWIN).all()
            ):
                return None
        w2 = np.maximum(w2, n2)
        plans.append({"rows": rows, "m1": m1, "m2": m2, "p1": p1, "p2": p2})
    return blo, ntiles, w2, plans


def _build_program(ntiles, blo, w2):
    import concourse.bacc as bacc
    import concourse.mybir as mybir
    from concourse.tile import TileContext

    f32, bf = mybir.dt.float32, mybir.dt.bfloat16
    u8 = mybir.dt.uint8
    off2 = np.concatenate([[0], np.cumsum(w2)])
    w2tot = int(off2[-1])
    nred = ntiles + int((w2 > 0).sum())
    ngrp = -(-nred // 3)

    nc = bacc.Bacc(None, target_bir_lowering=False)
    st_d = nc.dram_tensor(
        "stream", [ntiles, 128, NCHUNK * TILE], bf, kind="ExternalInput"
    )
    xc_d = nc.dram_tensor("xc", [128, NCHUNK * B], bf, kind="ExternalInput")
    mask_d = nc.dram_tensor("mask", [MWIN, ntiles * TILE], u8, kind="ExternalInput")
    mask2_d = nc.dram_tensor("mask2", [MWIN, max(w2tot, 1)], u8, kind="ExternalInput")
    ones_d = nc.dram_tensor("ones", [MWIN, 1], bf, kind="ExternalInput")
    out_d = nc.dram_tensor("out", [3, ngrp * TILE], f32, kind="ExternalOutput")

    MCH = 8  # mask chunk granularity (tiles) for just-in-time mask loads

    with TileContext(nc) as tc:
        with (
            tc.tile_pool(name="res", bufs=1) as res_pool,
            tc.tile_pool(name="g", bufs=14) as gpool,
            tc.tile_pool(name="m", bufs=8) as mpool,
            tc.tile_pool(name="o", bufs=4) as opool,
            tc.tile_pool(name="ps", bufs=5, space="PSUM") as pspool,
            tc.tile_pool(name="pso", bufs=2, space="PSUM") as psopool,
        ):
            xc_sb = res_pool.tile([128, NCHUNK * B], bf, tag="xc")
            nc.sync.dma_start(out=xc_sb[:], in_=xc_d[:])
            ones_sb = res_pool.tile([MWIN, 1], bf, tag="ones")
            nc.sync.dma_start(out=ones_sb[:], in_=ones_d[:])
            # mask/mask2 SBUF tiles are resident but loaded just-in-time in
            # MCH-tile chunks so the stream loads aren't stuck behind them.
            mask_sb = res_pool.tile([MWIN, ntiles * TILE], u8, tag="mask")
            mask2_sb = res_pool.tile([MWIN, max(w2tot, 1)], u8, tag="mask2")

            DELAY = 3  # pending masked tiles between DVE mask-mult and PE reduce
            pending = []
            state = {"r": 0, "bank": None}

            def emit_reduce(msk_u, width):
                r = state["r"]
                q, row = divmod(r, 3)
                if row == 0:
                    state["bank"] = psopool.tile(
                        [128, TILE], f32, tag="pso", name="pso_bank"
                    )
                bank = state["bank"]
                nc.tensor.matmul(
                    out=bank[32 * row : 32 * row + 1, :width],
                    lhsT=ones_sb[:],
                    rhs=msk_u[:, :width],
                    start=True,
                    stop=True,
                )
                if row == 2 or r == nred - 1:
                    # Ship this bank with one ACT copy of partitions 0..64
                    # (only rows 0/32/64 carry results; compute engines
                    # cannot stride partitions, DMA can).
                    nrow = row + 1
                    span = 32 * (nrow - 1) + 1
                    ot = opool.tile([65, TILE], f32, tag="ot", name="ot_buf")
                    nc.scalar.copy(ot[:span, :], bank[:span, :])
                    nc.scalar.dma_start(
                        out=out_d[:nrow, q * TILE : (q + 1) * TILE],
                        in_=ot[:span:32, :],
                    )
                state["r"] = r + 1

            for t in range(ntiles):
                bl = int(blo[t])
                if t % MCH == 0:
                    # JIT mask chunk for tiles [t, t+MCH)
                    lo, hi = t * TILE, min(ntiles, t + MCH) * TILE
                    nc.sync.dma_start(
                        out=mask_sb[:, lo:hi], in_=mask_d[:, lo:hi]
                    )
                    l2, h2 = int(off2[t]), int(off2[min(ntiles, t + MCH)])
                    if h2 > l2:
                        nc.scalar.dma_start(
                            out=mask2_sb[:, l2:h2], in_=mask2_d[:, l2:h2]
                        )
                g = gpool.tile([128, NCHUNK * TILE], bf, tag="g")
                eng = nc.sync if t % 2 == 0 else nc.scalar
                eng.dma_start(out=g[:], in_=st_d[t])
                ps = pspool.tile([MWIN, TILE], f32, tag="ps")
                for c in range(NCHUNK):
                    nc.tensor.matmul(
                        out=ps[:],
                        lhsT=xc_sb[:, c * B + bl : c * B + bl + MWIN],
                        rhs=g[:, c * TILE : (c + 1) * TILE],
                        start=(c == 0),
                        stop=(c == NCHUNK - 1),
                    )
                while len(pending) > DELAY:
                    emit_reduce(*pending.pop(0))
                msk = mpool.tile([MWIN, TILE], bf, tag="msk")
                nc.vector.tensor_tensor(
                    out=msk[:],
                    in0=ps[:],
                    in1=mask_sb[:, t * TILE : (t + 1) * TILE],
                    op=mybir.AluOpType.mult,
                )
                pending.append((msk, TILE))
                wt = int(w2[t])
                if wt > 0:
                    # Pass 2: serves each column's second hit — the 2-hit
                    # columns sit first, so only a prefix is touched.
                    msk2 = mpool.tile([MWIN, TILE], bf, tag="msk2")
                    nc.vector.tensor_tensor(
                        out=msk2[:, :wt],
                        in0=ps[:, :wt],
                        in1=mask2_sb[:, int(off2[t]) : int(off2[t]) + wt],
                        op=mybir.AluOpType.mult,
                    )
                    pending.append((msk2, wt))
            for args in pending:
                emit_reduce(*args)

    nc.compile()
    return nc


def _prep_inputs(input, labels, weight, alpha, beta, shortlist, cap=CAP):
    """Host-side staging: gate fold into a single [L,512] bf16 table, segment
    packing, per-tile pre-transpose, mask build."""
    input = np.asarray(input, dtype=np.float32)
    alpha = np.asarray(alpha, dtype=np.float32).reshape(1, D)
    beta = np.asarray(beta, dtype=np.float32).reshape(1, D)
    sa = 1.0 / (1.0 + np.exp(-alpha))
    sb = 1.0 / (1.0 + np.exp(-beta))

    # XC[p, c, b]: chunk c of input for batch b.
    XC = np.ascontiguousarray(
        input.T.reshape(NCHUNK, 128, B).transpose(1, 0, 2)
    ).astype(BF16)

    TC = (
        np.asarray(weight, np.float32) * sa + np.asarray(labels, np.float32) * sb
    ).astype(BF16)  # [L, 512]

    sl = np.asarray(shortlist).reshape(-1).astype(np.int64)
    built = _build_plans(sl, cap)
    if built is None:
        return None
    blo, ntiles, w2, plans = built
    off2 = np.concatenate([[0], np.cumsum(w2)])
    w2tot = int(off2[-1])

    in_maps = []
    ones = np.ones((MWIN, 1), dtype=BF16)
    xc_flat = np.ascontiguousarray(XC.reshape(128, NCHUNK * B))
    for c in range(NCORES):
        pl = plans[c]
        arr = TC[c * LSH : (c + 1) * LSH][pl["rows"].reshape(-1)]
        arr = arr.reshape(ntiles, TILE, NCHUNK, 128)      # [t, j, ch, p]
        stream = np.ascontiguousarray(arr.transpose(0, 3, 2, 1)).reshape(
            ntiles, 128, NCHUNK * TILE
        )
        maskh = np.zeros((MWIN, ntiles * TILE), np.uint8)
        mask2h = np.zeros((MWIN, max(w2tot, 1)), np.uint8)
        tt, jj = np.nonzero(pl["m1"] >= 0)
        maskh[pl["m1"][tt, jj], tt * TILE + jj] = 1
        tt, jj = np.nonzero(pl["m2"] >= 0)
        mask2h[pl["m2"][tt, jj], off2[tt] + jj] = 1
        in_maps.append(
            {
                "stream": stream,
                "xc": xc_flat,
                "mask": maskh,
                "mask2": mask2h,
                "ones": ones,
            }
        )
    meta = {"blo": blo, "ntiles": ntiles, "w2": w2, "off2": off2, "plans": plans}
    return in_maps, meta


def kernel(input, labels, weight, alpha, beta, bias, shortlist, _trace=False):
    from concourse.bass_utils import run_bass_kernel_spmd

    prep = _prep_inputs(input, labels, weight, alpha, beta, shortlist)
    if prep is None:
        # Window invariant violated (cannot happen by construction, but keep
        # a safe fallback): one hit per column, pure batch-sorted packing.
        prep = _prep_inputs(input, labels, weight, alpha, beta, shortlist, cap=1)
    assert prep is not None, "segment packing failed"
    in_maps, meta = prep
    ntiles, w2, blo = meta["ntiles"], meta["w2"], meta["blo"]

    key = (ntiles, tuple(int(x) for x in w2), tuple(int(x) for x in blo))
    if key not in _PROG_CACHE:
        _PROG_CACHE[key] = _build_program(ntiles, blo, w2)
    nc = _PROG_CACHE[key]

    res = run_bass_kernel_spmd(nc, in_maps, list(range(NCORES)), trace=_trace)

    # Reduce r (emission order: per tile, pass1 then pass2-if-any) lives at
    # out[r%3, (r//3)*TILE : +TILE].
    r1 = np.empty(ntiles, np.int64)
    r2 = np.full(ntiles, -1, np.int64)
    r = 0
    for t in range(ntiles):
        r1[t] = r
        r += 1
        if w2[t] > 0:
            r2[t] = r
            r += 1

    out_flat = np.zeros(B * S, dtype=np.float32)
    for c in range(NCORES):
        o = res.results[c]["out"]  # [3, ngrp*TILE]
        pl = meta["plans"][c]
        for t in range(ntiles):
            q, row = divmod(int(r1[t]), 3)
            vals = o[row, q * TILE : (q + 1) * TILE]
            sel = pl["p1"][t] >= 0
            out_flat[pl["p1"][t][sel]] = vals[sel]
            if r2[t] >= 0:
                q, row = divmod(int(r2[t]), 3)
                vals = o[row, q * TILE : (q + 1) * TILE]
                sel = pl["p2"][t] >= 0
                out_flat[pl["p2"][t][sel]] = vals[sel]

    bias = np.asarray(bias, dtype=np.float32)
    sl = np.asarray(shortlist).reshape(-1).astype(np.int64)
    out_flat += bias[sl]
    out = out_flat.reshape(B, S)

    if _trace:
        return out, res
    return out
